# revision 3
# baseline (speedup 1.0000x reference)
"""Trainium2 Bass kernel for MEAttention (sparse_attention), 8-core data parallel.

Layout strategy (per core, 4 samples):
  - Work in transposed layout [C, N] (channel on partitions) which is x's
    native layout and the output layout; softmax-over-channels (q) handled
    via Exp + deferred row-sum normalization applied at the very end
    (everything after q is linear in q per token, and both branches share
    the same 1/rowsum factor).
  - softmax-over-tokens (keys, branch k) never needs a max/partition
    reduction: values are O(0.3) so exp is safe unnormalized; the
    normalizer comes from appending a ones-column to V in the ctx matmul.
  - srN convs (stride==kernel, non-overlapping patches) are computed as 64
    (resp 16) shift-matmuls accumulating in PSUM, batched over all 4
    samples in the free dimension.
  - Per-channel biases on free-dim layouts: bk/bkv[k-half] cancel in
    token-softmax; bv shifts ctx by a constant (softmax sums to 1);
    bq is a per-partition Exp bias; rp/rp12/dw are folded on the host.

Wire-format optimizations (wall time is transfer-dominated, not compute):
  - Every large tensor crosses the host<->device tunnel in bf16: x, all
    matmul/conv weights, and the output (upcast to fp32 host-side).
  - The two big conv weights (sr1 16.8MB, sr2 4.2MB fp32) are sharded
    1/8-per-core over the wire and reassembled on device with an HBM
    AllGather across the 8 cores, instead of being replicated 8x.
  - All heavy matmuls run with bf16 operands (fp32 PSUM accumulate);
    normalization/LayerNorm paths stay fp32.
"""

import sys

if "/opt/trn_rl_repo" not in sys.path:
    sys.path.insert(0, "/opt/trn_rl_repo")

import numpy as np
import ml_dtypes

B, C, H, W = 32, 256, 56, 56
N = H * W  # 3136
Ch = C // 2  # 128
NCORES = 8
SPC = B // NCORES  # 4 samples per core
NCHUNK = 448  # 3136 = 7*448, fits one PSUM bank (fp32 <=512)
NCH = N // NCHUNK  # 7

BF16 = ml_dtypes.bfloat16

_compiled = None


def _build():
    import concourse.bass as bass
    import concourse.bacc as bacc
    import concourse.mybir as mybir
    import concourse.tile as tile
    from concourse.masks import make_identity

    dt = mybir.dt.float32
    bt = mybir.dt.bfloat16
    AF = mybir.ActivationFunctionType
    OP = mybir.AluOpType
    AX = mybir.AxisListType

    nc = bacc.Bacc(
        "TRN2", target_bir_lowering=False, debug=False, num_devices=NCORES
    )

    def din(name, shape, d=bt):
        return nc.dram_tensor(name, shape, d, kind="ExternalInput").ap()

    x4 = din("x4", [SPC, C, H, W])
    wq_d = din("wq", [C, C])
    bq_d = din("bq_col", [C, 1], dt)
    wkv_d = din("wkv_cat", [C, 2 * C])
    bv_d = din("bv_b", [128, C], dt)
    wkv1_d = din("wkv1", [C, C])
    wkv2_d = din("wkv2", [C, C])
    bkv1v_d = din("bkv1v_col", [Ch, 1], dt)
    bkv2v_d = din("bkv2v_col", [Ch, 1], dt)
    sr1s_d = din("sr1_shard", [8, C, C])
    sr1b_d = din("sr1_b_col", [C, 1], dt)
    sr2s_d = din("sr2_shard", [2, C, C])
    sr2b_d = din("sr2_b_col", [C, 1], dt)
    g1_d = din("g1_b", [128, C], dt)
    b1_d = din("b1_b", [128, C], dt)
    g2_d = din("g2_b", [128, C], dt)
    b2_d = din("b2_b", [128, C], dt)
    lc1w_d = din("lc1_w9", [Ch, 9], dt)
    lc1b_d = din("lc1_b_col", [Ch, 1], dt)
    lc2w_d = din("lc2_w9", [Ch, 9], dt)
    lc2b_d = din("lc2_b_col", [Ch, 1], dt)
    rpw_d = din("rpw2t", [C, C])
    rp12w_d = din("rp12w2t", [C, C])
    rpb_d = din("rpb2_col", [C, 1], dt)

    out4 = nc.dram_tensor("out4", [SPC, C, H, W], bt, kind="ExternalOutput").ap()

    with tile.TileContext(nc) as tc:
        import contextlib

        es = contextlib.ExitStack()
        with es:
            dram = es.enter_context(tc.tile_pool(name="dram", bufs=1, space="DRAM"))
            const = es.enter_context(tc.tile_pool(name="const", bufs=1))
            xpool = es.enter_context(tc.tile_pool(name="xp", bufs=1))
            persist = es.enter_context(tc.tile_pool(name="persist", bufs=1))
            convw = es.enter_context(tc.tile_pool(name="convw", bufs=4))
            brs = es.enter_context(tc.tile_pool(name="brs", bufs=2))
            enp = es.enter_context(tc.tile_pool(name="enp", bufs=2))
            chp = es.enter_context(tc.tile_pool(name="chp", bufs=2))

            # ---- AllGather the sharded conv weights (HBM -> HBM) ----
            sr1_bin = dram.tile([8, C, C], bt, name="sr1_bin", tag="sr1_bin")
            sr1_full = dram.tile(
                [64, C, C], bt, name="sr1_full", tag="sr1_full", addr_space="Shared"
            )
            sr2_bin = dram.tile([2, C, C], bt, name="sr2_bin", tag="sr2_bin")
            sr2_full = dram.tile(
                [16, C, C], bt, name="sr2_full", tag="sr2_full", addr_space="Shared"
            )
            nc.gpsimd.dma_start(sr1_bin[:], sr1s_d[:])
            nc.gpsimd.dma_start(sr2_bin[:], sr2s_d[:])
            rg = [list(range(NCORES))]
            nc.gpsimd.collective_compute(
                "AllGather",
                mybir.AluOpType.bypass,
                replica_groups=rg,
                ins=[sr1_bin.opt()],
                outs=[sr1_full.opt()],
            )
            nc.gpsimd.collective_compute(
                "AllGather",
                mybir.AluOpType.bypass,
                replica_groups=rg,
                ins=[sr2_bin.opt()],
                outs=[sr2_full.opt()],
            )

            # ---- constants / weights ----
            ident = const.tile([128, 128], dt)
            make_identity(nc, ident[:])
            ones_col = const.tile([128, 1], bt)
            nc.gpsimd.memset(ones_col[:], 1.0)
            ones_row = const.tile([1, 128], dt)
            nc.gpsimd.memset(ones_row[:], 1.0)
            eps_col = const.tile([128, 1], dt)
            nc.gpsimd.memset(eps_col[:], 1e-5)

            def load2(src, cols, tag, d=bt):
                ts_ = []
                for ct in range(2):
                    t = const.tile([128, cols], d, name=f"{tag}{ct}", tag=f"{tag}{ct}")
                    nc.sync.dma_start(t[:], src[128 * ct : 128 * (ct + 1), :])
                    ts_.append(t)
                return ts_

            wq_sb = load2(wq_d, C, "wq")
            wkv_sb = load2(wkv_d, 2 * C, "wkv")
            wkv1_sb = load2(wkv1_d, C, "wkv1")
            wkv2_sb = load2(wkv2_d, C, "wkv2")
            rpw_sb = load2(rpw_d, C, "rpw")
            rp12w_sb = load2(rp12w_d, C, "rp12w")
            bq_sb = load2(bq_d, 1, "bq", dt)
            sr1b_sb = load2(sr1b_d, 1, "sr1b", dt)
            sr2b_sb = load2(sr2b_d, 1, "sr2b", dt)
            rpb_sb = load2(rpb_d, 1, "rpb", dt)

            def load1(src, shape, tag, d=dt):
                t = const.tile(shape, d, tag=tag)
                nc.sync.dma_start(t[:], src[:])
                return t

            bv_sb = load1(bv_d, [128, C], "bv")
            g1_sb = load1(g1_d, [128, C], "g1")
            b1_sb = load1(b1_d, [128, C], "b1")
            g2_sb = load1(g2_d, [128, C], "g2")
            b2_sb = load1(b2_d, [128, C], "b2")
            lc1w_sb = load1(lc1w_d, [Ch, 9], "lc1w")
            lc1b_sb = load1(lc1b_d, [Ch, 1], "lc1b")
            lc2w_sb = load1(lc2w_d, [Ch, 9], "lc2w")
            lc2b_sb = load1(lc2b_d, [Ch, 1], "lc2b")
            bkv1v_sb = load1(bkv1v_d, [Ch, 1], "bkv1v")
            bkv2v_sb = load1(bkv2v_d, [Ch, 1], "bkv2v")

            # ---- X resident: [128, SPC*N] per channel-half (bf16) ----
            xall = []
            for ct in range(2):
                t = xpool.tile([128, SPC * N], bt, name=f"xall{ct}", tag=f"xall{ct}")
                for s in range(SPC):
                    nc.sync.dma_start(
                        t[:, s * N : (s + 1) * N],
                        x4[s, 128 * ct : 128 * (ct + 1)].rearrange(
                            "c h w -> c (h w)"
                        ),
                    )
                xall.append(t)

            # ================= PHASE A: spatial-reduction convs =================
            conv_psum = tc.tile_pool(name="cpsum", bufs=1, space="PSUM")
            cps = conv_psum.__enter__()
            # sr1: stride 8, 8x8 kernel -> 7x7=49 tokens/sample, 196 batched
            x1p = [cps.tile([128, 4 * 49], dt, name=f"x1p{ot}", tag=f"x1p{ot}") for ot in range(2)]
            for j in range(64):
                dy, dx = j // 8, j % 8
                for ct in range(2):
                    wt = convw.tile([128, C], bt, name="cw", tag="cw")
                    nc.sync.dma_start(
                        wt[:], sr1_full[j, 128 * ct : 128 * (ct + 1), :]
                    )
                    rr = xall[ct][:].rearrange(
                        "p (sy yi xo xi) -> p sy yi xo xi", sy=28, yi=8, xo=7, xi=8
                    )
                    rhs = rr[:, :, dy, :, dx]
                    for ot in range(2):
                        nc.tensor.matmul(
                            x1p[ot][:],
                            wt[:, 128 * ot : 128 * (ot + 1)],
                            rhs,
                            start=(j == 0 and ct == 0),
                            stop=(j == 63 and ct == 1),
                        )
            x1c = []
            for ot in range(2):
                t = persist.tile([128, 4 * 49], dt, name=f"x1c{ot}", tag=f"x1c{ot}")
                nc.scalar.activation(t[:], x1p[ot][:], AF.Identity, bias=sr1b_sb[ot][:])
                x1c.append(t)

            # sr2: stride 4, 4x4 kernel -> 14x14=196 tokens/sample, 784 batched
            # split (s,py)=56 rows into 2 halves of 28 -> free 28*14=392
            x2p = [
                [cps.tile([128, 392], dt, name=f"x2p{h}{ot}", tag=f"x2p{h}{ot}") for ot in range(2)]
                for h in range(2)
            ]
            for j in range(16):
                dy, dx = j // 4, j % 4
                for ct in range(2):
                    wt = convw.tile([128, C], bt, name="cw", tag="cw")
                    nc.sync.dma_start(
                        wt[:], sr2_full[j, 128 * ct : 128 * (ct + 1), :]
                    )
                    rr = xall[ct][:].rearrange(
                        "p (sy yi xo xi) -> p sy yi xo xi", sy=56, yi=4, xo=14, xi=4
                    )
                    for h in range(2):
                        rhs = rr[:, 28 * h : 28 * (h + 1), dy, :, dx]
                        for ot in range(2):
                            nc.tensor.matmul(
                                x2p[h][ot][:],
                                wt[:, 128 * ot : 128 * (ot + 1)],
                                rhs,
                                start=(j == 0 and ct == 0),
                                stop=(j == 15 and ct == 1),
                            )
            x2c = []
            for ot in range(2):
                t = persist.tile([128, 4 * 196], dt, name=f"x2c{ot}", tag=f"x2c{ot}")
                for h in range(2):
                    nc.scalar.activation(
                        t[:, 392 * h : 392 * (h + 1)],
                        x2p[h][ot][:],
                        AF.Identity,
                        bias=sr2b_sb[ot][:],
                    )
                x2c.append(t)

            conv_psum.__exit__(None, None, None)

            # ---- per-sample branch processing (tiny) ----
            def layer_norm(xt, p, g_sb, b_sb, out):
                # xt: [p, 256] sbuf; out: [p, 256] post-LN+GELU
                mu = brs.tile([128, 1], dt, name="ln_mu", tag="ln_mu")
                nc.vector.reduce_sum(mu[:p, :], xt, axis=AX.X)
                nc.scalar.mul(mu[:p, :], mu[:p, :], 1.0 / C)
                xc = brs.tile([128, C], dt, name="ln_xc", tag="ln_xc", bufs=1)
                nc.vector.tensor_scalar(
                    xc[:p, :], xt, mu[:p, :], None, op0=OP.subtract
                )
                sq = brs.tile([128, C], dt, name="ln_sq", tag="ln_sq", bufs=1)
                nc.scalar.square(sq[:p, :], xc[:p, :])
                var = brs.tile([128, 1], dt, name="ln_var", tag="ln_var")
                nc.vector.reduce_sum(var[:p, :], sq[:p, :], axis=AX.X)
                std = brs.tile([128, 1], dt, name="ln_std", tag="ln_std")
                nc.scalar.activation(
                    std[:p, :], var[:p, :], AF.Sqrt, bias=eps_col[:p, :], scale=1.0 / C
                )
                rstd = brs.tile([128, 1], dt, name="ln_rstd", tag="ln_rstd")
                nc.vector.reciprocal(rstd[:p, :], std[:p, :])
                xn = brs.tile([128, C], dt, name="ln_xn", tag="ln_xn", bufs=1)
                nc.vector.tensor_scalar(
                    xn[:p, :], xc[:p, :], rstd[:p, :], None, op0=OP.mult
                )
                t2 = brs.tile([128, C], dt, name="ln_t2", tag="ln_t2", bufs=1)
                nc.vector.tensor_mul(t2[:p, :], xn[:p, :], g_sb[:p, :])
                t3 = brs.tile([128, C], dt, name="ln_t3", tag="ln_t3", bufs=1)
                nc.vector.tensor_add(t3[:p, :], t2[:p, :], b_sb[:p, :])
                nc.scalar.activation(out, t3[:p, :], AF.Gelu)

            def dw_conv(vtb, hh, lcw_sb, lcb_sb, tagp):
                # vtb: [128, hh*hh] sbuf (channel-major); returns (acc+lcb)+vtb
                pad = hh + 2
                vpad = brs.tile([128, pad * pad], dt, name=f"{tagp}_pad", tag=f"{tagp}_pad")
                nc.gpsimd.memset(vpad[:], 0.0)
                pv = vpad[:].rearrange("p (y x) -> p y x", y=pad, x=pad)
                nc.vector.tensor_copy(
                    pv[:, 1 : hh + 1, 1 : hh + 1],
                    vtb.rearrange("p (y x) -> p y x", y=hh, x=hh),
                )
                acc = None
                for j in range(9):
                    dy, dx = j // 3, j % 3
                    src = pv[:, dy : dy + hh, dx : dx + hh]
                    nacc = brs.tile([128, hh * hh], dt, name=f"{tagp}_acc{j % 2}", tag=f"{tagp}_acc{j % 2}")
                    if acc is None:
                        nc.vector.tensor_scalar(
                            nacc[:], src, lcw_sb[:, j : j + 1], None, op0=OP.mult
                        )
                    else:
                        nc.vector.scalar_tensor_tensor(
                            nacc[:],
                            src,
                            lcw_sb[:, j : j + 1],
                            acc[:],
                            op0=OP.mult,
                            op1=OP.add,
                        )
                    acc = nacc
                vfull = brs.tile([128, hh * hh], dt, name=f"{tagp}_vf", tag=f"{tagp}_vf")
                nc.vector.scalar_tensor_tensor(
                    vfull[:], acc[:], lcb_sb[:], vtb, op0=OP.add, op1=OP.add
                )
                return vfull

            br_tp = tc.tile_pool(name="tpp", bufs=2, space="PSUM")
            tpp = br_tp.__enter__()
            br_bp = tc.tile_pool(name="bps", bufs=2, space="PSUM")
            bps = br_bp.__enter__()
            ctx1n = []
            ctx2n = []
            for s in range(SPC):
                # ---------- branch 1 (49 tokens) ----------
                x1t = brs.tile([49, C], dt, name="x1t", tag="x1t")
                for ct in range(2):
                    pt = tpp.tile([49, 128], dt, name="tp_a", tag="tp_a")
                    nc.tensor.transpose(
                        pt[:], x1c[ct][:, 49 * s : 49 * (s + 1)], ident[:]
                    )
                    nc.vector.tensor_copy(x1t[:, 128 * ct : 128 * (ct + 1)], pt[:])
                x1n = brs.tile([49, C], dt, name="x1n", tag="x1n")
                layer_norm(x1t[:], 49, g1_sb, b1_sb, x1n[:])
                kv1p = bps.tile([49, C], dt, name="kv1p", tag="kvbr")
                for ct in range(2):
                    pt = tpp.tile([128, 49], dt, name="tp_b", tag="tp_b")
                    nc.tensor.transpose(
                        pt[:], x1n[:, 128 * ct : 128 * (ct + 1)], ident[:49, :49]
                    )
                    x1nT = brs.tile([128, 49], bt, name="x1nT", tag="x1nT")
                    nc.vector.tensor_copy(x1nT[:], pt[:])
                    nc.tensor.matmul(
                        kv1p[:],
                        x1nT[:],
                        wkv1_sb[ct][:],
                        start=(ct == 0),
                        stop=(ct == 1),
                    )
                e1 = brs.tile([49, Ch], bt, name="e1", tag="e1")
                nc.scalar.activation(e1[:], kv1p[:, 0:Ch], AF.Exp)
                v1s = brs.tile([49, Ch], dt, name="v1s", tag="v1s")
                nc.vector.tensor_copy(v1s[:], kv1p[:, Ch : 2 * Ch])
                ptv = tpp.tile([128, 49], dt, name="tp_b", tag="tp_b")
                nc.tensor.transpose(ptv[:], v1s[:], ident[:49, :49])
                v1tb = brs.tile([128, 49], dt, name="v1tb", tag="v1tb")
                nc.vector.tensor_scalar(
                    v1tb[:], ptv[:], bkv1v_sb[:], None, op0=OP.add
                )
                v1full = dw_conv(v1tb[:], 7, lc1w_sb, lc1b_sb, "c1")
                ptb = tpp.tile([49, 128], dt, name="tp_a", tag="tp_a")
                nc.tensor.transpose(ptb[:], v1full[:], ident[:])
                v1e = brs.tile([49, Ch + 1], bt, name="v1e", tag="v1e")
                nc.gpsimd.memset(v1e[:, Ch : Ch + 1], 1.0)
                nc.vector.tensor_copy(v1e[:, 0:Ch], ptb[:])
                c1p = bps.tile([128, Ch + 1], dt, name="c1p", tag="cbr")
                nc.tensor.matmul(c1p[:], e1[:], v1e[:], start=True, stop=True)
                s1i = brs.tile([128, 1], dt, name="s1i", tag="s1i")
                nc.vector.reciprocal(s1i[:], c1p[:, Ch : Ch + 1])
                c1n = persist.tile([128, Ch], bt, name=f"ctx1n{s}", tag=f"ctx1n{s}")
                nc.vector.tensor_scalar(
                    c1n[:], c1p[:, 0:Ch], s1i[:], None, op0=OP.mult
                )
                ctx1n.append(c1n)

                # ---------- branch 2 (196 tokens: chunks 128+68) ----------
                x2t_a = brs.tile([128, C], dt, name="x2t_a", tag="x2t_a")
                x2t_b = brs.tile([68, C], dt, name="x2t_b", tag="x2t_b")
                for ct in range(2):
                    pt = tpp.tile([128, 128], dt, name="tp_a", tag="tp_a")
                    nc.tensor.transpose(
                        pt[:], x2c[ct][:, 196 * s : 196 * s + 128], ident[:]
                    )
                    nc.vector.tensor_copy(x2t_a[:, 128 * ct : 128 * (ct + 1)], pt[:])
                    pt2 = tpp.tile([68, 128], dt, name="tp_a", tag="tp_a")
                    nc.tensor.transpose(
                        pt2[:], x2c[ct][:, 196 * s + 128 : 196 * (s + 1)], ident[:]
                    )
                    nc.vector.tensor_copy(
                        x2t_b[:, 128 * ct : 128 * (ct + 1)], pt2[:]
                    )
                x2n_a = brs.tile([128, C], dt, name="x2n_a", tag="x2n_a")
                x2n_b = brs.tile([68, C], dt, name="x2n_b", tag="x2n_b")
                layer_norm(x2t_a[:], 128, g2_sb, b2_sb, x2n_a[:])
                layer_norm(x2t_b[:], 68, g2_sb, b2_sb, x2n_b[:])
                kv2pa = bps.tile([128, C], dt, name="kv2pa", tag="kvbr")
                kv2pb = bps.tile([68, C], dt, name="kv2pb", tag="kvbr")
                for ct in range(2):
                    pt = tpp.tile([128, 128], dt, name="tp_b", tag="tp_b")
                    nc.tensor.transpose(
                        pt[:], x2n_a[:, 128 * ct : 128 * (ct + 1)], ident[:]
                    )
                    x2nTa = brs.tile([128, 128], bt, name="x2nTa", tag="x2nTa")
                    nc.vector.tensor_copy(x2nTa[:], pt[:])
                    nc.tensor.matmul(
                        kv2pa[:],
                        x2nTa[:],
                        wkv2_sb[ct][:],
                        start=(ct == 0),
                        stop=(ct == 1),
                    )
                    pt2 = tpp.tile([128, 68], dt, name="tp_b", tag="tp_b")
                    nc.tensor.transpose(
                        pt2[:], x2n_b[:, 128 * ct : 128 * (ct + 1)], ident[:68, :68]
                    )
                    x2nTb = brs.tile([128, 68], bt, name="x2nTb", tag="x2nTb")
                    nc.vector.tensor_copy(x2nTb[:], pt2[:])
                    nc.tensor.matmul(
                        kv2pb[:],
                        x2nTb[:],
                        wkv2_sb[ct][:],
                        start=(ct == 0),
                        stop=(ct == 1),
                    )
                e2a = brs.tile([128, Ch], bt, name="e2a", tag="e2a")
                e2b = brs.tile([68, Ch], bt, name="e2b", tag="e2b")
                nc.scalar.activation(e2a[:], kv2pa[:, 0:Ch], AF.Exp)
                nc.scalar.activation(e2b[:], kv2pb[:, 0:Ch], AF.Exp)
                v2sa = brs.tile([128, Ch], dt, name="v2sa", tag="v2sa")
                v2sb_ = brs.tile([68, Ch], dt, name="v2sb", tag="v2sb")
                nc.vector.tensor_copy(v2sa[:], kv2pa[:, Ch : 2 * Ch])
                nc.vector.tensor_copy(v2sb_[:], kv2pb[:, Ch : 2 * Ch])
                v2tb = brs.tile([128, 196], dt, name="v2tb", tag="v2tb")
                ptva = tpp.tile([128, 128], dt, name="tp_b", tag="tp_b")
                nc.tensor.transpose(ptva[:], v2sa[:], ident[:])
                nc.vector.tensor_scalar(
                    v2tb[:, 0:128], ptva[:], bkv2v_sb[:], None, op0=OP.add
                )
                ptvb = tpp.tile([128, 68], dt, name="tp_b", tag="tp_b")
                nc.tensor.transpose(ptvb[:], v2sb_[:], ident[:68, :68])
                nc.vector.tensor_scalar(
                    v2tb[:, 128:196], ptvb[:], bkv2v_sb[:], None, op0=OP.add
                )
                v2full = dw_conv(v2tb[:], 14, lc2w_sb, lc2b_sb, "c2")
                v2e_a = brs.tile([128, Ch + 1], bt, name="v2e_a", tag="v2e_a")
                v2e_b = brs.tile([68, Ch + 1], bt, name="v2e_b", tag="v2e_b")
                pba = tpp.tile([128, 128], dt, name="tp_a", tag="tp_a")
                nc.tensor.transpose(pba[:], v2full[:, 0:128], ident[:])
                nc.gpsimd.memset(v2e_a[:, Ch : Ch + 1], 1.0)
                nc.vector.tensor_copy(v2e_a[:, 0:Ch], pba[:])
                pbb = tpp.tile([68, 128], dt, name="tp_a", tag="tp_a")
                nc.tensor.transpose(pbb[:], v2full[:, 128:196], ident[:])
                nc.gpsimd.memset(v2e_b[:, Ch : Ch + 1], 1.0)
                nc.vector.tensor_copy(v2e_b[:, 0:Ch], pbb[:])
                c2p = bps.tile([128, Ch + 1], dt, name="c2p", tag="cbr")
                nc.tensor.matmul(c2p[:], e2a[:], v2e_a[:], start=True, stop=False)
                nc.tensor.matmul(c2p[:], e2b[:], v2e_b[:], start=False, stop=True)
                s2i = brs.tile([128, 1], dt, name="s2i", tag="s2i")
                nc.vector.reciprocal(s2i[:], c2p[:, Ch : Ch + 1])
                c2n = persist.tile([128, Ch], bt, name=f"ctx2n{s}", tag=f"ctx2n{s}")
                nc.vector.tensor_scalar(
                    c2n[:], c2p[:, 0:Ch], s2i[:], None, op0=OP.mult
                )
                ctx2n.append(c2n)

            br_bp.__exit__(None, None, None)
            br_tp.__exit__(None, None, None)

            # ================= PHASE B: global attention per sample =============
            for s in range(SPC):
                # ---- ctx over all tokens: ctx[k,v] = sum_n exp(K)[n,k]*Vext[n,v]
                kv_ps = tc.tile_pool(name=f"kvps{s}", bufs=2, space="PSUM")
                kvp_pool = kv_ps.__enter__()
                ctx_ps = tc.tile_pool(name=f"ctxps{s}", bufs=1, space="PSUM")
                ctxp_pool = ctx_ps.__enter__()
                ctxp = [
                    ctxp_pool.tile([128, C + 1], dt, name=f"ctxp{kt}", tag=f"ctxp{kt}")
                    for kt in range(2)
                ]
                for nt in range(25):
                    n0 = 128 * nt
                    sz = 64 if nt == 24 else 128
                    kvt = kvp_pool.tile([128, 2 * C], dt, name="kvt", tag="kvt")
                    for ct in range(2):
                        nc.tensor.matmul(
                            kvt[:sz, :],
                            xall[ct][:, s * N + n0 : s * N + n0 + sz],
                            wkv_sb[ct][:],
                            start=(ct == 0),
                            stop=(ct == 1),
                        )
                    en = enp.tile([128, C], bt, name="en", tag="en")
                    nc.scalar.activation(en[:sz, :], kvt[:sz, 0:C], AF.Exp)
                    vne = enp.tile([128, C + 1], bt, name="vne", tag="vne")
                    nc.gpsimd.memset(vne[:sz, C : C + 1], 1.0)
                    nc.vector.tensor_copy(vne[:sz, 0:C], kvt[:sz, C : 2 * C])
                    for kt in range(2):
                        nc.tensor.matmul(
                            ctxp[kt][:],
                            en[:sz, 128 * kt : 128 * (kt + 1)],
                            vne[:sz, :],
                            start=(nt == 0),
                            stop=(nt == 24),
                        )
                ctxg = []
                for kt in range(2):
                    si = brs.tile([128, 1], dt, name=f"gsi{kt}", tag=f"gsi{kt}")
                    nc.vector.reciprocal(si[:], ctxp[kt][:, C : C + 1])
                    cg = persist.tile([128, C], bt, name=f"ctxg{kt}", tag=f"ctxg{kt}")
                    nc.vector.scalar_tensor_tensor(
                        cg[:],
                        ctxp[kt][:, 0:C],
                        si[:],
                        bv_sb[:],
                        op0=OP.mult,
                        op1=OP.add,
                    )
                    ctxg.append(cg)

                ctx_ps.__exit__(None, None, None)
                kv_ps.__exit__(None, None, None)
                ch_ps = tc.tile_pool(name=f"chps{s}", bufs=2, space="PSUM")
                chpp = ch_ps.__enter__()

                # ---- per n-chunk: q, rs, att, a1, a2, project, combine, store
                for chk in range(NCH):
                    c0 = s * N + NCHUNK * chk
                    eq = []
                    for ct in range(2):
                        qp = chpp.tile([128, NCHUNK], dt, name="qp", tag="qp")
                        for kt in range(2):
                            nc.tensor.matmul(
                                qp[:],
                                wq_sb[kt][:, 128 * ct : 128 * (ct + 1)],
                                xall[kt][:, c0 : c0 + NCHUNK],
                                start=(kt == 0),
                                stop=(kt == 1),
                            )
                        et = chp.tile([128, NCHUNK], bt, name=f"eq{ct}", tag=f"eq{ct}")
                        nc.scalar.activation(
                            et[:], qp[:], AF.Exp, bias=bq_sb[ct][:]
                        )
                        eq.append(et)
                    # row-sum of exp(q) over channels -> 1/rs, broadcast to 128p
                    rsp = chpp.tile([1, NCHUNK], dt, name="rsp", tag="rsp", bufs=1)
                    for ct in range(2):
                        nc.tensor.matmul(
                            rsp[:],
                            ones_col[:],
                            eq[ct][:],
                            start=(ct == 0),
                            stop=(ct == 1),
                        )
                    rsi = chp.tile([1, NCHUNK], dt, name="rsi", tag="rsi")
                    nc.vector.reciprocal(rsi[:], rsp[:])
                    bc = chpp.tile([128, NCHUNK], dt, name="bc", tag="bc", bufs=1)
                    nc.tensor.matmul(bc[:], ones_row[:], rsi[:], start=True, stop=True)
                    bcs = chp.tile([128, NCHUNK], dt, name="bcs", tag="bcs", bufs=1)
                    nc.scalar.copy(bcs[:], bc[:])

                    att = []
                    for ot in range(2):
                        ab = chpp.tile([128, NCHUNK], dt, name="attp", tag="attp")
                        for kt in range(2):
                            nc.tensor.matmul(
                                ab[:],
                                ctxg[kt][:, 128 * ot : 128 * (ot + 1)],
                                eq[kt][:],
                                start=(kt == 0),
                                stop=(kt == 1),
                            )
                        ac = chp.tile([128, NCHUNK], bt, name=f"attc{ot}", tag=f"attc{ot}", bufs=1)
                        nc.scalar.copy(ac[:], ab[:])
                        att.append(ac)
                    a1b = chpp.tile([128, NCHUNK], dt, name="attp", tag="attp")
                    nc.tensor.matmul(
                        a1b[:], ctx1n[s][:], eq[0][:], start=True, stop=True
                    )
                    a1c = chp.tile([128, NCHUNK], bt, name="a1c", tag="a1c", bufs=1)
                    nc.vector.tensor_copy(a1c[:], a1b[:])
                    a2b = chpp.tile([128, NCHUNK], dt, name="attp", tag="attp")
                    nc.tensor.matmul(
                        a2b[:], ctx2n[s][:], eq[1][:], start=True, stop=True
                    )
                    a2c = chp.tile([128, NCHUNK], bt, name="a2c", tag="a2c", bufs=1)
                    nc.vector.tensor_copy(a2c[:], a2b[:])

                    for ot in range(2):
                        osl = slice(128 * ot, 128 * (ot + 1))
                        op_ = chpp.tile([128, NCHUNK], dt, name="outp", tag="outp")
                        nc.tensor.matmul(
                            op_[:], rpw_sb[0][:, osl], att[0][:], start=True, stop=False
                        )
                        nc.tensor.matmul(
                            op_[:], rpw_sb[1][:, osl], att[1][:], start=False, stop=False
                        )
                        nc.tensor.matmul(
                            op_[:], rp12w_sb[0][:, osl], a1c[:], start=False, stop=False
                        )
                        nc.tensor.matmul(
                            op_[:], rp12w_sb[1][:, osl], a2c[:], start=False, stop=True
                        )
                        t = chp.tile([128, NCHUNK], dt, name=f"fin{ot}", tag=f"fin{ot}", bufs=1)
                        nc.vector.tensor_mul(t[:], op_[:], bcs[:])
                        f2 = chp.tile([128, NCHUNK], bt, name=f"fin2{ot}", tag=f"fin2{ot}", bufs=1)
                        nc.scalar.activation(
                            f2[:], t[:], AF.Identity, bias=rpb_sb[ot][:]
                        )
                        nc.sync.dma_start(
                            out4[s, osl].rearrange("c h w -> c (h w)")[
                                :, NCHUNK * chk : NCHUNK * (chk + 1)
                            ],
                            f2[:],
                        )
                ch_ps.__exit__(None, None, None)

    nc.compile()
    return nc


def _prep_inputs(inputs):
    f32 = np.float32

    def a(x):
        return np.ascontiguousarray(np.asarray(x, dtype=f32))

    def b(x):
        return np.ascontiguousarray(np.asarray(x, dtype=f32)).astype(BF16)

    Wq, bq = a(inputs["Wq"]), a(inputs["bq"])
    Wk, Wv = a(inputs["Wk"]), a(inputs["Wv"])
    bv = a(inputs["bv"])
    dw = a(inputs["dw_w"])
    dw0, dw1 = dw[:, 0], dw[:, 1]
    rp_w, rp_b = a(inputs["rp_w"]), a(inputs["rp_b"])
    rp12_w, rp12_b = a(inputs["rp12_w"]), a(inputs["rp12_b"])

    sr1_wt = a(inputs["sr1_w"]).transpose(2, 3, 1, 0).reshape(64, C, C).astype(BF16)
    sr2_wt = a(inputs["sr2_w"]).transpose(2, 3, 1, 0).reshape(16, C, C).astype(BF16)

    com = {
        "wq": Wq.astype(BF16),
        "bq_col": bq.reshape(C, 1).copy(),
        "wkv_cat": np.concatenate([Wk, Wv], axis=1).astype(BF16),
        "bv_b": np.broadcast_to(bv, (128, C)).copy(),
        "wkv1": b(inputs["Wkv1"]),
        "wkv2": b(inputs["Wkv2"]),
        "bkv1v_col": a(inputs["bkv1"])[Ch:].reshape(Ch, 1).copy(),
        "bkv2v_col": a(inputs["bkv2"])[Ch:].reshape(Ch, 1).copy(),
        "sr1_b_col": a(inputs["sr1_b"]).reshape(C, 1).copy(),
        "sr2_b_col": a(inputs["sr2_b"]).reshape(C, 1).copy(),
        "g1_b": np.broadcast_to(a(inputs["ln1_g"]), (128, C)).copy(),
        "b1_b": np.broadcast_to(a(inputs["ln1_b"]), (128, C)).copy(),
        "g2_b": np.broadcast_to(a(inputs["ln2_g"]), (128, C)).copy(),
        "b2_b": np.broadcast_to(a(inputs["ln2_b"]), (128, C)).copy(),
        "lc1_w9": a(inputs["lc1_w"]).reshape(Ch, 9).copy(),
        "lc1_b_col": a(inputs["lc1_b"]).reshape(Ch, 1).copy(),
        "lc2_w9": a(inputs["lc2_w"]).reshape(Ch, 9).copy(),
        "lc2_b_col": a(inputs["lc2_b"]).reshape(Ch, 1).copy(),
        "rpw2t": (rp_w * dw0[:, None]).T.astype(BF16),
        "rp12w2t": (rp12_w * dw1[:, None]).T.astype(BF16),
        "rpb2_col": (rp_b * dw0 + rp12_b * dw1).reshape(C, 1).copy(),
    }
    x = np.asarray(inputs["x"], dtype=f32).astype(BF16)
    in_maps = []
    for c in range(NCORES):
        m = dict(com)
        m["x4"] = np.ascontiguousarray(x[SPC * c : SPC * (c + 1)])
        m["sr1_shard"] = np.ascontiguousarray(sr1_wt[8 * c : 8 * (c + 1)])
        m["sr2_shard"] = np.ascontiguousarray(sr2_wt[2 * c : 2 * (c + 1)])
        in_maps.append(m)
    return in_maps


def _run(inputs, trace=False):
    global _compiled
    if _compiled is None:
        _compiled = _build()
    from concourse import bass_utils

    in_maps = _prep_inputs(inputs)
    res = bass_utils.run_bass_kernel_spmd(
        _compiled, in_maps, core_ids=list(range(NCORES)), trace=trace
    )
    out = np.empty((B, C, H, W), np.float32)
    for c in range(NCORES):
        out[SPC * c : SPC * (c + 1)] = res.results[c]["out4"].astype(np.float32)
    return out, res


def kernel(**inputs):
    out, _ = _run(inputs, trace=False)
    return out


def kernel_timed(**inputs):
    out, res = _run(inputs, trace=True)
    return out, res


# revision 11
# speedup vs baseline: 1.2086x; 1.2086x over previous
"""Trainium2 Bass kernel for MEAttention (sparse_attention), 8-core data parallel.

Layout strategy (per core, 4 samples):
  - Work in transposed layout [C, N] (channel on partitions) which is x's
    native layout and the output layout; softmax-over-channels (q) handled
    via Exp + deferred row-sum normalization applied at the very end
    (everything after q is linear in q per token, and both branches share
    the same 1/rowsum factor).
  - softmax-over-tokens (keys, branch k) never needs a max/partition
    reduction: values are O(0.3) so exp is safe unnormalized; the
    normalizer comes from appending a ones-column to V in the ctx matmul.
  - srN convs (stride==kernel, non-overlapping patches) are computed as 64
    (resp 16) shift-matmuls accumulating in PSUM, batched over all 4
    samples in the free dimension.
  - Per-channel biases on free-dim layouts: bk/bkv[k-half] cancel in
    token-softmax; bv shifts ctx by a constant (softmax sums to 1);
    bq is a per-partition Exp bias; rp/rp12/dw are folded on the host.

Wire-format optimizations (wall time is transfer-dominated, not compute):
  - Every large tensor crosses the host<->device tunnel in bf16: x, all
    matmul/conv weights, and the output (upcast to fp32 host-side).
  - The two big conv weights (sr1 16.8MB, sr2 4.2MB fp32) are sharded
    1/8-per-core over the wire and reassembled on device with an HBM
    AllGather across the 8 cores, instead of being replicated 8x.
  - All heavy matmuls run with bf16 operands (fp32 PSUM accumulate);
    normalization/LayerNorm paths stay fp32.
"""

import sys

if "/opt/trn_rl_repo" not in sys.path:
    sys.path.insert(0, "/opt/trn_rl_repo")

import numpy as np
import ml_dtypes

B, C, H, W = 32, 256, 56, 56
N = H * W  # 3136
Ch = C // 2  # 128
NCORES = 8
SPC = B // NCORES  # 4 samples per core
NCHUNK = 448  # 3136 = 7*448, fits one PSUM bank (fp32 <=512)
NCH = N // NCHUNK  # 7

BF16 = ml_dtypes.bfloat16

# column offsets in the replicated bf16 weight pack [128, WREP_COLS]:
# each [C,C] matrix is stored as two [128,C] row-halves side by side;
# wkv_cat is [C,2C] -> two [128,2C] halves.
WOFF = {
    "wq": (0, C),
    "wkv": (512, 2 * C),
    "wkv1": (1536, C),
    "wkv2": (2048, C),
    "rpw": (2560, C),
    "rp12w": (3072, C),
}
WREP_COLS = 3584

# column offsets in the replicated fp32 smalls pack [128, SREP_COLS]:
# [C,1] vectors as two adjacent [128,1] cols; [128,C] broadcasts as-is.
SOFF = {
    "bq": 0,
    "sr1b": 2,
    "sr2b": 4,
    "rpb": 6,
    "lc1b": 8,
    "lc2b": 9,
    "bkv1v": 10,
    "bkv2v": 11,
    "lc1w9": 12,
    "lc2w9": 21,
    "bv": 30,
    "g1": 286,
    "b1": 542,
    "g2": 798,
    "b2": 1054,
}
SREP_COLS = 1310

_compiled = None


def _build():
    import concourse.bass as bass
    import concourse.bacc as bacc
    import concourse.mybir as mybir
    import concourse.tile as tile
    from concourse.masks import make_identity

    dt = mybir.dt.float32
    bt = mybir.dt.bfloat16
    AF = mybir.ActivationFunctionType
    OP = mybir.AluOpType
    AX = mybir.AxisListType

    nc = bacc.Bacc(
        "TRN2", target_bir_lowering=False, debug=False, num_devices=NCORES
    )

    def din(name, shape, d=bt):
        return nc.dram_tensor(name, shape, d, kind="ExternalInput").ap()

    # 4 packed inputs: per-array host->device transfer has ~85ms fixed cost
    # over the axon tunnel, so everything is packed into few arrays.
    x4 = din("x4", [SPC, C, H, W])
    wsh_d = din("wsh", [10, C, C])  # per-core shard: sr1_wt[8c:8c+8] + sr2_wt[2c:2c+2]
    wrep_d = din("wrep", [128, WREP_COLS])  # replicated bf16 weight pack
    srep_d = din("srep", [128, SREP_COLS], dt)  # replicated fp32 smalls pack

    out4 = nc.dram_tensor("out4", [SPC, C, H, W], bt, kind="ExternalOutput").ap()

    with tile.TileContext(nc) as tc:
        import contextlib

        es = contextlib.ExitStack()
        with es:
            dram = es.enter_context(tc.tile_pool(name="dram", bufs=1, space="DRAM"))
            const = es.enter_context(tc.tile_pool(name="const", bufs=1))
            xpool = es.enter_context(tc.tile_pool(name="xp", bufs=1))
            persist = es.enter_context(tc.tile_pool(name="persist", bufs=1))
            convw = es.enter_context(tc.tile_pool(name="convw", bufs=4))
            brs = es.enter_context(tc.tile_pool(name="brs", bufs=2))
            enp = es.enter_context(tc.tile_pool(name="enp", bufs=2))
            chp = es.enter_context(tc.tile_pool(name="chp", bufs=2))

            # ---- AllGather the sharded conv weights (HBM -> HBM) ----
            # gathered layout: core c contributes planes [10c..10c+10) =
            # sr1_wt[8c:8c+8] ++ sr2_wt[2c:2c+2]
            wsh_bin = dram.tile([10, C, C], bt, name="wsh_bin", tag="wsh_bin")
            wsh_full = dram.tile(
                [80, C, C], bt, name="wsh_full", tag="wsh_full", addr_space="Shared"
            )
            nc.gpsimd.dma_start(wsh_bin[:], wsh_d[:])
            rg = [list(range(NCORES))]
            nc.gpsimd.collective_compute(
                "AllGather",
                mybir.AluOpType.bypass,
                replica_groups=rg,
                ins=[wsh_bin.opt()],
                outs=[wsh_full.opt()],
            )

            wsh_ap = wsh_full[:]

            def sr1_plane(j):
                return wsh_ap[10 * (j // 8) + (j % 8)]

            def sr2_plane(j):
                return wsh_ap[10 * (j // 2) + 8 + (j % 2)]

            # ---- constants / weights ----
            ident = const.tile([128, 128], dt)
            make_identity(nc, ident[:])
            ones_col = const.tile([128, 1], bt)
            nc.gpsimd.memset(ones_col[:], 1.0)
            ones_row = const.tile([1, 128], dt)
            nc.gpsimd.memset(ones_row[:], 1.0)
            eps_col = const.tile([128, 1], dt)
            nc.gpsimd.memset(eps_col[:], 1e-5)

            # single-DMA loads of the two replicated packs; everything else
            # is an AP slice into these SBUF tiles.
            wrep_t = const.tile([128, WREP_COLS], bt, name="wrep_t", tag="wrep_t")
            nc.sync.dma_start(wrep_t[:], wrep_d[:])
            srep_t = const.tile([128, SREP_COLS], dt, name="srep_t", tag="srep_t")
            nc.sync.dma_start(srep_t[:], srep_d[:])

            def w2(key):
                off, cols = WOFF[key]
                return [wrep_t[:, off + cols * ct : off + cols * (ct + 1)] for ct in range(2)]

            def s2(key):
                off = SOFF[key]
                return [srep_t[:, off + ct : off + ct + 1] for ct in range(2)]

            wq_sb = w2("wq")
            wkv_sb = w2("wkv")
            wkv1_sb = w2("wkv1")
            wkv2_sb = w2("wkv2")
            rpw_sb = w2("rpw")
            rp12w_sb = w2("rp12w")
            bq_sb = s2("bq")
            sr1b_sb = s2("sr1b")
            sr2b_sb = s2("sr2b")
            rpb_sb = s2("rpb")

            def s1(key, cols):
                off = SOFF[key]
                return srep_t[:, off : off + cols]

            bv_sb = s1("bv", C)
            g1_sb = s1("g1", C)
            b1_sb = s1("b1", C)
            g2_sb = s1("g2", C)
            b2_sb = s1("b2", C)
            lc1w_sb = s1("lc1w9", 9)
            lc1b_sb = s1("lc1b", 1)
            lc2w_sb = s1("lc2w9", 9)
            lc2b_sb = s1("lc2b", 1)
            bkv1v_sb = s1("bkv1v", 1)
            bkv2v_sb = s1("bkv2v", 1)

            # ---- X resident: [128, SPC*N] per channel-half (bf16) ----
            xall = []
            for ct in range(2):
                t = xpool.tile([128, SPC * N], bt, name=f"xall{ct}", tag=f"xall{ct}")
                for s in range(SPC):
                    nc.sync.dma_start(
                        t[:, s * N : (s + 1) * N],
                        x4[s, 128 * ct : 128 * (ct + 1)].rearrange(
                            "c h w -> c (h w)"
                        ),
                    )
                xall.append(t)

            # ================= PHASE A: spatial-reduction convs =================
            conv_psum = tc.tile_pool(name="cpsum", bufs=1, space="PSUM")
            cps = conv_psum.__enter__()
            # sr1: stride 8, 8x8 kernel -> 7x7=49 tokens/sample, 196 batched
            x1p = [cps.tile([128, 4 * 49], dt, name=f"x1p{ot}", tag=f"x1p{ot}") for ot in range(2)]
            for j in range(64):
                dy, dx = j // 8, j % 8
                for ct in range(2):
                    wt = convw.tile([128, C], bt, name="cw", tag="cw")
                    nc.sync.dma_start(
                        wt[:], sr1_plane(j)[128 * ct : 128 * (ct + 1), :]
                    )
                    rr = xall[ct][:].rearrange(
                        "p (sy yi xo xi) -> p sy yi xo xi", sy=28, yi=8, xo=7, xi=8
                    )
                    rhs = rr[:, :, dy, :, dx]
                    for ot in range(2):
                        nc.tensor.matmul(
                            x1p[ot][:],
                            wt[:, 128 * ot : 128 * (ot + 1)],
                            rhs,
                            start=(j == 0 and ct == 0),
                            stop=(j == 63 and ct == 1),
                        )
            x1c = []
            for ot in range(2):
                t = persist.tile([128, 4 * 49], dt, name=f"x1c{ot}", tag=f"x1c{ot}")
                nc.scalar.activation(t[:], x1p[ot][:], AF.Identity, bias=sr1b_sb[ot][:])
                x1c.append(t)

            # sr2: stride 4, 4x4 kernel -> 14x14=196 tokens/sample, 784 batched
            # split (s,py)=56 rows into 2 halves of 28 -> free 28*14=392
            x2p = [
                [cps.tile([128, 392], dt, name=f"x2p{h}{ot}", tag=f"x2p{h}{ot}") for ot in range(2)]
                for h in range(2)
            ]
            for j in range(16):
                dy, dx = j // 4, j % 4
                for ct in range(2):
                    wt = convw.tile([128, C], bt, name="cw", tag="cw")
                    nc.sync.dma_start(
                        wt[:], sr2_plane(j)[128 * ct : 128 * (ct + 1), :]
                    )
                    rr = xall[ct][:].rearrange(
                        "p (sy yi xo xi) -> p sy yi xo xi", sy=56, yi=4, xo=14, xi=4
                    )
                    for h in range(2):
                        rhs = rr[:, 28 * h : 28 * (h + 1), dy, :, dx]
                        for ot in range(2):
                            nc.tensor.matmul(
                                x2p[h][ot][:],
                                wt[:, 128 * ot : 128 * (ot + 1)],
                                rhs,
                                start=(j == 0 and ct == 0),
                                stop=(j == 15 and ct == 1),
                            )
            x2c = []
            for ot in range(2):
                t = persist.tile([128, 4 * 196], dt, name=f"x2c{ot}", tag=f"x2c{ot}")
                for h in range(2):
                    nc.scalar.activation(
                        t[:, 392 * h : 392 * (h + 1)],
                        x2p[h][ot][:],
                        AF.Identity,
                        bias=sr2b_sb[ot][:],
                    )
                x2c.append(t)

            conv_psum.__exit__(None, None, None)

            # ---- per-sample branch processing (tiny) ----
            def layer_norm(xt, p, g_sb, b_sb, out):
                # xt: [p, 256] sbuf; out: [p, 256] post-LN+GELU
                mu = brs.tile([128, 1], dt, name="ln_mu", tag="ln_mu")
                nc.vector.reduce_sum(mu[:p, :], xt, axis=AX.X)
                nc.scalar.mul(mu[:p, :], mu[:p, :], 1.0 / C)
                xc = brs.tile([128, C], dt, name="ln_xc", tag="ln_xc", bufs=1)
                nc.vector.tensor_scalar(
                    xc[:p, :], xt, mu[:p, :], None, op0=OP.subtract
                )
                sq = brs.tile([128, C], dt, name="ln_sq", tag="ln_sq", bufs=1)
                nc.scalar.square(sq[:p, :], xc[:p, :])
                var = brs.tile([128, 1], dt, name="ln_var", tag="ln_var")
                nc.vector.reduce_sum(var[:p, :], sq[:p, :], axis=AX.X)
                std = brs.tile([128, 1], dt, name="ln_std", tag="ln_std")
                nc.scalar.activation(
                    std[:p, :], var[:p, :], AF.Sqrt, bias=eps_col[:p, :], scale=1.0 / C
                )
                rstd = brs.tile([128, 1], dt, name="ln_rstd", tag="ln_rstd")
                nc.vector.reciprocal(rstd[:p, :], std[:p, :])
                xn = brs.tile([128, C], dt, name="ln_xn", tag="ln_xn", bufs=1)
                nc.vector.tensor_scalar(
                    xn[:p, :], xc[:p, :], rstd[:p, :], None, op0=OP.mult
                )
                t2 = brs.tile([128, C], dt, name="ln_t2", tag="ln_t2", bufs=1)
                nc.vector.tensor_mul(t2[:p, :], xn[:p, :], g_sb[:p, :])
                t3 = brs.tile([128, C], dt, name="ln_t3", tag="ln_t3", bufs=1)
                nc.vector.tensor_add(t3[:p, :], t2[:p, :], b_sb[:p, :])
                nc.scalar.activation(out, t3[:p, :], AF.Gelu)

            def dw_conv(vtb, hh, lcw_sb, lcb_sb, tagp):
                # vtb: [128, hh*hh] sbuf (channel-major); returns (acc+lcb)+vtb
                pad = hh + 2
                vpad = brs.tile([128, pad * pad], dt, name=f"{tagp}_pad", tag=f"{tagp}_pad")
                nc.gpsimd.memset(vpad[:], 0.0)
                pv = vpad[:].rearrange("p (y x) -> p y x", y=pad, x=pad)
                nc.vector.tensor_copy(
                    pv[:, 1 : hh + 1, 1 : hh + 1],
                    vtb.rearrange("p (y x) -> p y x", y=hh, x=hh),
                )
                acc = None
                for j in range(9):
                    dy, dx = j // 3, j % 3
                    src = pv[:, dy : dy + hh, dx : dx + hh]
                    nacc = brs.tile([128, hh * hh], dt, name=f"{tagp}_acc{j % 2}", tag=f"{tagp}_acc{j % 2}")
                    if acc is None:
                        nc.vector.tensor_scalar(
                            nacc[:], src, lcw_sb[:, j : j + 1], None, op0=OP.mult
                        )
                    else:
                        nc.vector.scalar_tensor_tensor(
                            nacc[:],
                            src,
                            lcw_sb[:, j : j + 1],
                            acc[:],
                            op0=OP.mult,
                            op1=OP.add,
                        )
                    acc = nacc
                vfull = brs.tile([128, hh * hh], dt, name=f"{tagp}_vf", tag=f"{tagp}_vf")
                nc.vector.scalar_tensor_tensor(
                    vfull[:], acc[:], lcb_sb[:], vtb, op0=OP.add, op1=OP.add
                )
                return vfull

            br_tp = tc.tile_pool(name="tpp", bufs=2, space="PSUM")
            tpp = br_tp.__enter__()
            br_bp = tc.tile_pool(name="bps", bufs=2, space="PSUM")
            bps = br_bp.__enter__()
            ctx1n = []
            ctx2n = []
            for s in range(SPC):
                # ---------- branch 1 (49 tokens) ----------
                x1t = brs.tile([49, C], dt, name="x1t", tag="x1t")
                for ct in range(2):
                    pt = tpp.tile([49, 128], dt, name="tp_a", tag="tp_a")
                    nc.tensor.transpose(
                        pt[:], x1c[ct][:, 49 * s : 49 * (s + 1)], ident[:]
                    )
                    nc.vector.tensor_copy(x1t[:, 128 * ct : 128 * (ct + 1)], pt[:])
                x1n = brs.tile([49, C], dt, name="x1n", tag="x1n")
                layer_norm(x1t[:], 49, g1_sb, b1_sb, x1n[:])
                kv1p = bps.tile([49, C], dt, name="kv1p", tag="kvbr")
                for ct in range(2):
                    pt = tpp.tile([128, 49], dt, name="tp_b", tag="tp_b")
                    nc.tensor.transpose(
                        pt[:], x1n[:, 128 * ct : 128 * (ct + 1)], ident[:49, :49]
                    )
                    x1nT = brs.tile([128, 49], bt, name="x1nT", tag="x1nT")
                    nc.vector.tensor_copy(x1nT[:], pt[:])
                    nc.tensor.matmul(
                        kv1p[:],
                        x1nT[:],
                        wkv1_sb[ct][:],
                        start=(ct == 0),
                        stop=(ct == 1),
                    )
                e1 = brs.tile([49, Ch], bt, name="e1", tag="e1")
                nc.scalar.activation(e1[:], kv1p[:, 0:Ch], AF.Exp)
                v1s = brs.tile([49, Ch], dt, name="v1s", tag="v1s")
                nc.vector.tensor_copy(v1s[:], kv1p[:, Ch : 2 * Ch])
                ptv = tpp.tile([128, 49], dt, name="tp_b", tag="tp_b")
                nc.tensor.transpose(ptv[:], v1s[:], ident[:49, :49])
                v1tb = brs.tile([128, 49], dt, name="v1tb", tag="v1tb")
                nc.vector.tensor_scalar(
                    v1tb[:], ptv[:], bkv1v_sb[:], None, op0=OP.add
                )
                v1full = dw_conv(v1tb[:], 7, lc1w_sb, lc1b_sb, "c1")
                ptb = tpp.tile([49, 128], dt, name="tp_a", tag="tp_a")
                nc.tensor.transpose(ptb[:], v1full[:], ident[:])
                v1e = brs.tile([49, Ch + 1], bt, name="v1e", tag="v1e")
                nc.gpsimd.memset(v1e[:, Ch : Ch + 1], 1.0)
                nc.vector.tensor_copy(v1e[:, 0:Ch], ptb[:])
                c1p = bps.tile([128, Ch + 1], dt, name="c1p", tag="cbr")
                nc.tensor.matmul(c1p[:], e1[:], v1e[:], start=True, stop=True)
                s1i = brs.tile([128, 1], dt, name="s1i", tag="s1i")
                nc.vector.reciprocal(s1i[:], c1p[:, Ch : Ch + 1])
                c1n = persist.tile([128, Ch], bt, name=f"ctx1n{s}", tag=f"ctx1n{s}")
                nc.vector.tensor_scalar(
                    c1n[:], c1p[:, 0:Ch], s1i[:], None, op0=OP.mult
                )
                ctx1n.append(c1n)

                # ---------- branch 2 (196 tokens: chunks 128+68) ----------
                x2t_a = brs.tile([128, C], dt, name="x2t_a", tag="x2t_a")
                x2t_b = brs.tile([68, C], dt, name="x2t_b", tag="x2t_b")
                for ct in range(2):
                    pt = tpp.tile([128, 128], dt, name="tp_a", tag="tp_a")
                    nc.tensor.transpose(
                        pt[:], x2c[ct][:, 196 * s : 196 * s + 128], ident[:]
                    )
                    nc.vector.tensor_copy(x2t_a[:, 128 * ct : 128 * (ct + 1)], pt[:])
                    pt2 = tpp.tile([68, 128], dt, name="tp_a", tag="tp_a")
                    nc.tensor.transpose(
                        pt2[:], x2c[ct][:, 196 * s + 128 : 196 * (s + 1)], ident[:]
                    )
                    nc.vector.tensor_copy(
                        x2t_b[:, 128 * ct : 128 * (ct + 1)], pt2[:]
                    )
                x2n_a = brs.tile([128, C], dt, name="x2n_a", tag="x2n_a")
                x2n_b = brs.tile([68, C], dt, name="x2n_b", tag="x2n_b")
                layer_norm(x2t_a[:], 128, g2_sb, b2_sb, x2n_a[:])
                layer_norm(x2t_b[:], 68, g2_sb, b2_sb, x2n_b[:])
                kv2pa = bps.tile([128, C], dt, name="kv2pa", tag="kvbr")
                kv2pb = bps.tile([68, C], dt, name="kv2pb", tag="kvbr")
                for ct in range(2):
                    pt = tpp.tile([128, 128], dt, name="tp_b", tag="tp_b")
                    nc.tensor.transpose(
                        pt[:], x2n_a[:, 128 * ct : 128 * (ct + 1)], ident[:]
                    )
                    x2nTa = brs.tile([128, 128], bt, name="x2nTa", tag="x2nTa")
                    nc.vector.tensor_copy(x2nTa[:], pt[:])
                    nc.tensor.matmul(
                        kv2pa[:],
                        x2nTa[:],
                        wkv2_sb[ct][:],
                        start=(ct == 0),
                        stop=(ct == 1),
                    )
                    pt2 = tpp.tile([128, 68], dt, name="tp_b", tag="tp_b")
                    nc.tensor.transpose(
                        pt2[:], x2n_b[:, 128 * ct : 128 * (ct + 1)], ident[:68, :68]
                    )
                    x2nTb = brs.tile([128, 68], bt, name="x2nTb", tag="x2nTb")
                    nc.vector.tensor_copy(x2nTb[:], pt2[:])
                    nc.tensor.matmul(
                        kv2pb[:],
                        x2nTb[:],
                        wkv2_sb[ct][:],
                        start=(ct == 0),
                        stop=(ct == 1),
                    )
                e2a = brs.tile([128, Ch], bt, name="e2a", tag="e2a")
                e2b = brs.tile([68, Ch], bt, name="e2b", tag="e2b")
                nc.scalar.activation(e2a[:], kv2pa[:, 0:Ch], AF.Exp)
                nc.scalar.activation(e2b[:], kv2pb[:, 0:Ch], AF.Exp)
                v2sa = brs.tile([128, Ch], dt, name="v2sa", tag="v2sa")
                v2sb_ = brs.tile([68, Ch], dt, name="v2sb", tag="v2sb")
                nc.vector.tensor_copy(v2sa[:], kv2pa[:, Ch : 2 * Ch])
                nc.vector.tensor_copy(v2sb_[:], kv2pb[:, Ch : 2 * Ch])
                v2tb = brs.tile([128, 196], dt, name="v2tb", tag="v2tb")
                ptva = tpp.tile([128, 128], dt, name="tp_b", tag="tp_b")
                nc.tensor.transpose(ptva[:], v2sa[:], ident[:])
                nc.vector.tensor_scalar(
                    v2tb[:, 0:128], ptva[:], bkv2v_sb[:], None, op0=OP.add
                )
                ptvb = tpp.tile([128, 68], dt, name="tp_b", tag="tp_b")
                nc.tensor.transpose(ptvb[:], v2sb_[:], ident[:68, :68])
                nc.vector.tensor_scalar(
                    v2tb[:, 128:196], ptvb[:], bkv2v_sb[:], None, op0=OP.add
                )
                v2full = dw_conv(v2tb[:], 14, lc2w_sb, lc2b_sb, "c2")
                v2e_a = brs.tile([128, Ch + 1], bt, name="v2e_a", tag="v2e_a")
                v2e_b = brs.tile([68, Ch + 1], bt, name="v2e_b", tag="v2e_b")
                pba = tpp.tile([128, 128], dt, name="tp_a", tag="tp_a")
                nc.tensor.transpose(pba[:], v2full[:, 0:128], ident[:])
                nc.gpsimd.memset(v2e_a[:, Ch : Ch + 1], 1.0)
                nc.vector.tensor_copy(v2e_a[:, 0:Ch], pba[:])
                pbb = tpp.tile([68, 128], dt, name="tp_a", tag="tp_a")
                nc.tensor.transpose(pbb[:], v2full[:, 128:196], ident[:])
                nc.gpsimd.memset(v2e_b[:, Ch : Ch + 1], 1.0)
                nc.vector.tensor_copy(v2e_b[:, 0:Ch], pbb[:])
                c2p = bps.tile([128, Ch + 1], dt, name="c2p", tag="cbr")
                nc.tensor.matmul(c2p[:], e2a[:], v2e_a[:], start=True, stop=False)
                nc.tensor.matmul(c2p[:], e2b[:], v2e_b[:], start=False, stop=True)
                s2i = brs.tile([128, 1], dt, name="s2i", tag="s2i")
                nc.vector.reciprocal(s2i[:], c2p[:, Ch : Ch + 1])
                c2n = persist.tile([128, Ch], bt, name=f"ctx2n{s}", tag=f"ctx2n{s}")
                nc.vector.tensor_scalar(
                    c2n[:], c2p[:, 0:Ch], s2i[:], None, op0=OP.mult
                )
                ctx2n.append(c2n)

            br_bp.__exit__(None, None, None)
            br_tp.__exit__(None, None, None)

            # ================= PHASE B: global attention per sample =============
            for s in range(SPC):
                # ---- ctx over all tokens: ctx[k,v] = sum_n exp(K)[n,k]*Vext[n,v]
                kv_ps = tc.tile_pool(name=f"kvps{s}", bufs=2, space="PSUM")
                kvp_pool = kv_ps.__enter__()
                ctx_ps = tc.tile_pool(name=f"ctxps{s}", bufs=1, space="PSUM")
                ctxp_pool = ctx_ps.__enter__()
                ctxp = [
                    ctxp_pool.tile([128, C + 1], dt, name=f"ctxp{kt}", tag=f"ctxp{kt}")
                    for kt in range(2)
                ]
                for nt in range(25):
                    n0 = 128 * nt
                    sz = 64 if nt == 24 else 128
                    kvt = kvp_pool.tile([128, 2 * C], dt, name="kvt", tag="kvt")
                    for ct in range(2):
                        nc.tensor.matmul(
                            kvt[:sz, :],
                            xall[ct][:, s * N + n0 : s * N + n0 + sz],
                            wkv_sb[ct][:],
                            start=(ct == 0),
                            stop=(ct == 1),
                        )
                    en = enp.tile([128, C], bt, name="en", tag="en")
                    nc.scalar.activation(en[:sz, :], kvt[:sz, 0:C], AF.Exp)
                    vne = enp.tile([128, C + 1], bt, name="vne", tag="vne")
                    nc.gpsimd.memset(vne[:sz, C : C + 1], 1.0)
                    nc.vector.tensor_copy(vne[:sz, 0:C], kvt[:sz, C : 2 * C])
                    for kt in range(2):
                        nc.tensor.matmul(
                            ctxp[kt][:],
                            en[:sz, 128 * kt : 128 * (kt + 1)],
                            vne[:sz, :],
                            start=(nt == 0),
                            stop=(nt == 24),
                        )
                ctxg = []
                for kt in range(2):
                    si = brs.tile([128, 1], dt, name=f"gsi{kt}", tag=f"gsi{kt}")
                    nc.vector.reciprocal(si[:], ctxp[kt][:, C : C + 1])
                    cg = persist.tile([128, C], bt, name=f"ctxg{kt}", tag=f"ctxg{kt}")
                    nc.vector.scalar_tensor_tensor(
                        cg[:],
                        ctxp[kt][:, 0:C],
                        si[:],
                        bv_sb[:],
                        op0=OP.mult,
                        op1=OP.add,
                    )
                    ctxg.append(cg)

                ctx_ps.__exit__(None, None, None)
                kv_ps.__exit__(None, None, None)
                ch_ps = tc.tile_pool(name=f"chps{s}", bufs=2, space="PSUM")
                chpp = ch_ps.__enter__()

                # ---- per n-chunk: q, rs, att, a1, a2, project, combine, store
                for chk in range(NCH):
                    c0 = s * N + NCHUNK * chk
                    eq = []
                    for ct in range(2):
                        qp = chpp.tile([128, NCHUNK], dt, name="qp", tag="qp")
                        for kt in range(2):
                            nc.tensor.matmul(
                                qp[:],
                                wq_sb[kt][:, 128 * ct : 128 * (ct + 1)],
                                xall[kt][:, c0 : c0 + NCHUNK],
                                start=(kt == 0),
                                stop=(kt == 1),
                            )
                        et = chp.tile([128, NCHUNK], bt, name=f"eq{ct}", tag=f"eq{ct}")
                        nc.scalar.activation(
                            et[:], qp[:], AF.Exp, bias=bq_sb[ct][:]
                        )
                        eq.append(et)
                    # row-sum of exp(q) over channels -> 1/rs, broadcast to 128p
                    rsp = chpp.tile([1, NCHUNK], dt, name="rsp", tag="rsp", bufs=1)
                    for ct in range(2):
                        nc.tensor.matmul(
                            rsp[:],
                            ones_col[:],
                            eq[ct][:],
                            start=(ct == 0),
                            stop=(ct == 1),
                        )
                    rsi = chp.tile([1, NCHUNK], dt, name="rsi", tag="rsi")
                    nc.vector.reciprocal(rsi[:], rsp[:])
                    bc = chpp.tile([128, NCHUNK], dt, name="bc", tag="bc", bufs=1)
                    nc.tensor.matmul(bc[:], ones_row[:], rsi[:], start=True, stop=True)
                    bcs = chp.tile([128, NCHUNK], dt, name="bcs", tag="bcs", bufs=1)
                    nc.scalar.copy(bcs[:], bc[:])

                    att = []
                    for ot in range(2):
                        ab = chpp.tile([128, NCHUNK], dt, name="attp", tag="attp")
                        for kt in range(2):
                            nc.tensor.matmul(
                                ab[:],
                                ctxg[kt][:, 128 * ot : 128 * (ot + 1)],
                                eq[kt][:],
                                start=(kt == 0),
                                stop=(kt == 1),
                            )
                        ac = chp.tile([128, NCHUNK], bt, name=f"attc{ot}", tag=f"attc{ot}", bufs=1)
                        nc.scalar.copy(ac[:], ab[:])
                        att.append(ac)
                    a1b = chpp.tile([128, NCHUNK], dt, name="attp", tag="attp")
                    nc.tensor.matmul(
                        a1b[:], ctx1n[s][:], eq[0][:], start=True, stop=True
                    )
                    a1c = chp.tile([128, NCHUNK], bt, name="a1c", tag="a1c", bufs=1)
                    nc.vector.tensor_copy(a1c[:], a1b[:])
                    a2b = chpp.tile([128, NCHUNK], dt, name="attp", tag="attp")
                    nc.tensor.matmul(
                        a2b[:], ctx2n[s][:], eq[1][:], start=True, stop=True
                    )
                    a2c = chp.tile([128, NCHUNK], bt, name="a2c", tag="a2c", bufs=1)
                    nc.vector.tensor_copy(a2c[:], a2b[:])

                    for ot in range(2):
                        osl = slice(128 * ot, 128 * (ot + 1))
                        op_ = chpp.tile([128, NCHUNK], dt, name="outp", tag="outp")
                        nc.tensor.matmul(
                            op_[:], rpw_sb[0][:, osl], att[0][:], start=True, stop=False
                        )
                        nc.tensor.matmul(
                            op_[:], rpw_sb[1][:, osl], att[1][:], start=False, stop=False
                        )
                        nc.tensor.matmul(
                            op_[:], rp12w_sb[0][:, osl], a1c[:], start=False, stop=False
                        )
                        nc.tensor.matmul(
                            op_[:], rp12w_sb[1][:, osl], a2c[:], start=False, stop=True
                        )
                        t = chp.tile([128, NCHUNK], dt, name=f"fin{ot}", tag=f"fin{ot}", bufs=1)
                        nc.vector.tensor_mul(t[:], op_[:], bcs[:])
                        f2 = chp.tile([128, NCHUNK], bt, name=f"fin2{ot}", tag=f"fin2{ot}", bufs=1)
                        nc.scalar.activation(
                            f2[:], t[:], AF.Identity, bias=rpb_sb[ot][:]
                        )
                        nc.sync.dma_start(
                            out4[s, osl].rearrange("c h w -> c (h w)")[
                                :, NCHUNK * chk : NCHUNK * (chk + 1)
                            ],
                            f2[:],
                        )
                ch_ps.__exit__(None, None, None)

    nc.compile()
    return nc


def _prep_inputs(inputs):
    f32 = np.float32

    def a(x):
        return np.ascontiguousarray(np.asarray(x, dtype=f32))

    def b(x):
        return np.ascontiguousarray(np.asarray(x, dtype=f32)).astype(BF16)

    Wq, bq = a(inputs["Wq"]), a(inputs["bq"])
    Wk, Wv = a(inputs["Wk"]), a(inputs["Wv"])
    bv = a(inputs["bv"])
    dw = a(inputs["dw_w"])
    dw0, dw1 = dw[:, 0], dw[:, 1]
    rp_w, rp_b = a(inputs["rp_w"]), a(inputs["rp_b"])
    rp12_w, rp12_b = a(inputs["rp12_w"]), a(inputs["rp12_b"])

    sr1_wt = a(inputs["sr1_w"]).transpose(2, 3, 1, 0).reshape(64, C, C).astype(BF16)
    sr2_wt = a(inputs["sr2_w"]).transpose(2, 3, 1, 0).reshape(16, C, C).astype(BF16)

    wrep = np.zeros((128, WREP_COLS), BF16)

    def put_w(key, mat):
        off, cols = WOFF[key]
        assert mat.shape == (C, cols)
        wrep[:, off : off + cols] = mat[:128]
        wrep[:, off + cols : off + 2 * cols] = mat[128:]

    put_w("wq", Wq.astype(BF16))
    put_w("wkv", np.concatenate([Wk, Wv], axis=1).astype(BF16))
    put_w("wkv1", b(inputs["Wkv1"]))
    put_w("wkv2", b(inputs["Wkv2"]))
    put_w("rpw", (rp_w * dw0[:, None]).T.astype(BF16))
    put_w("rp12w", (rp12_w * dw1[:, None]).T.astype(BF16))

    srep = np.zeros((128, SREP_COLS), f32)

    def put_col2(key, vec):
        off = SOFF[key]
        srep[:, off] = vec[:128]
        srep[:, off + 1] = vec[128:]

    put_col2("bq", bq)
    put_col2("sr1b", a(inputs["sr1_b"]))
    put_col2("sr2b", a(inputs["sr2_b"]))
    put_col2("rpb", rp_b * dw0 + rp12_b * dw1)
    srep[:, SOFF["lc1b"]] = a(inputs["lc1_b"])
    srep[:, SOFF["lc2b"]] = a(inputs["lc2_b"])
    srep[:, SOFF["bkv1v"]] = a(inputs["bkv1"])[Ch:]
    srep[:, SOFF["bkv2v"]] = a(inputs["bkv2"])[Ch:]
    srep[:, SOFF["lc1w9"] : SOFF["lc1w9"] + 9] = a(inputs["lc1_w"]).reshape(Ch, 9)
    srep[:, SOFF["lc2w9"] : SOFF["lc2w9"] + 9] = a(inputs["lc2_w"]).reshape(Ch, 9)
    srep[:, SOFF["bv"] : SOFF["bv"] + C] = bv[None, :]
    srep[:, SOFF["g1"] : SOFF["g1"] + C] = a(inputs["ln1_g"])[None, :]
    srep[:, SOFF["b1"] : SOFF["b1"] + C] = a(inputs["ln1_b"])[None, :]
    srep[:, SOFF["g2"] : SOFF["g2"] + C] = a(inputs["ln2_g"])[None, :]
    srep[:, SOFF["b2"] : SOFF["b2"] + C] = a(inputs["ln2_b"])[None, :]

    x = np.asarray(inputs["x"], dtype=f32).astype(BF16)
    in_maps = []
    for c in range(NCORES):
        m = {
            "wrep": wrep,
            "srep": srep,
            "x4": np.ascontiguousarray(x[SPC * c : SPC * (c + 1)]),
            "wsh": np.concatenate(
                [sr1_wt[8 * c : 8 * (c + 1)], sr2_wt[2 * c : 2 * (c + 1)]], axis=0
            ),
        }
        in_maps.append(m)
    return in_maps


def _run(inputs, trace=False):
    global _compiled
    if _compiled is None:
        _compiled = _build()
    from concourse import bass_utils

    in_maps = _prep_inputs(inputs)
    res = bass_utils.run_bass_kernel_spmd(
        _compiled, in_maps, core_ids=list(range(NCORES)), trace=trace
    )
    out = np.empty((B, C, H, W), np.float32)
    for c in range(NCORES):
        out[SPC * c : SPC * (c + 1)] = res.results[c]["out4"].astype(np.float32)
    return out, res


def kernel(**inputs):
    out, _ = _run(inputs, trace=False)
    return out


def kernel_timed(**inputs):
    out, res = _run(inputs, trace=True)
    return out, res
